# revision 5
# baseline (speedup 1.0000x reference)
"""ChannelGuidedAttn Trainium2 kernel.

Reference computation (per batch b):
    q  = x_pre[b]      reshaped (C, N),  C=512, N=H*W=4096
    kv = x_training[b] reshaped (C, N)
    energy[c,d] = <q[c,:], kv[d,:]>                      (C x C)
    att = softmax(max_d(energy) - energy, axis=-1)       == softmax(-energy)
        = exp(min_d(energy) - energy) / sum
    out = att @ kv  -> (C, H, W);  final softmax over W

Sharding: data-parallel over batch B=16 across 8 cores (2 batches/core).

Per-core kernel (Tile framework):
  - gemm1 needs both operands with n on partitions -> fp16 cast + xbar
    DMA-transpose (2-byte dtype) of q and kv.
  - precision: fp16 hi/lo split for gemm1 (energy = qh.kh + ql.kh + qh.kl
    accumulated in fp32 PSUM) -> ~8.5e-4 final absmax rel err.
  - gemm2 uses att^T (small DMA-transpose) against kv in natural layout.
  - final softmax over W=64 segments: exp (no max-subtract needed; |out|<~6),
    strided segment reduce, broadcast multiply.
"""

import sys

import numpy as np

for _p in ("/opt/trn_rl_repo", "/root/.axon_site/_ro/trn_rl_repo"):
    if _p not in sys.path:
        sys.path.append(_p)

B = 16
N_CORES = 8
B_PER_CORE = B // N_CORES
C = 512
H = 64
W = 64
N = H * W
CT = C // 128  # 4 c-tiles
NJ = N // 128  # 32 n-chunks of 128
NO = N // 512  # 8 output chunks of 512

G1_SPLIT = True  # fp16 hi/lo split for gemm1 (3 matmuls instead of 1)


def build_program(g1_split=G1_SPLIT):
    from contextlib import ExitStack

    import concourse.mybir as mybir
    import concourse.tile as tile
    from concourse import bacc

    f32 = mybir.dt.float32
    f16 = mybir.dt.float16
    Alu = mybir.AluOpType
    Act = mybir.ActivationFunctionType
    Axis = mybir.AxisListType

    nc = bacc.Bacc()
    xt = nc.declare_dram_parameter("xt", [B_PER_CORE, C, N], f32, isOutput=False)
    xp = nc.declare_dram_parameter("xp", [B_PER_CORE, C, N], f32, isOutput=False)
    out = nc.declare_dram_parameter("out", [B_PER_CORE, C, N], f32, isOutput=True)

    with tile.TileContext(nc) as tc, ExitStack() as ctx:
        raw = ctx.enter_context(tc.tile_pool(name="raw", bufs=2))
        stage16 = ctx.enter_context(tc.tile_pool(name="stage16", bufs=3))
        kvpool = ctx.enter_context(tc.tile_pool(name="kvpool", bufs=1))
        qtpool = ctx.enter_context(tc.tile_pool(name="qtpool", bufs=2))
        att_pool = ctx.enter_context(tc.tile_pool(name="att", bufs=2))
        small = ctx.enter_context(tc.tile_pool(name="small", bufs=4))
        opool = ctx.enter_context(tc.tile_pool(name="opool", bufs=3))
        ps_e = ctx.enter_context(tc.tile_pool(name="ps_e", bufs=2, space="PSUM"))
        ps_o = ctx.enter_context(tc.tile_pool(name="ps_o", bufs=4, space="PSUM"))

        for b in range(B_PER_CORE):
            # ---- kv prep: natural fp16 + transposed fp16 (hi and lo) ----
            kh_nat = kvpool.tile([128, CT, N], f16, tag="kh_nat")
            khT = kvpool.tile([128, CT, NJ, 128], f16, tag="khT")
            if g1_split:
                klT = kvpool.tile([128, CT, NJ, 128], f16, tag="klT")
            for dt in range(CT):
                kv_f32 = raw.tile([128, N], f32, tag="raw")
                nc.sync.dma_start(out=kv_f32, in_=xt[b, dt * 128 : (dt + 1) * 128, :])
                nc.gpsimd.tensor_copy(out=kh_nat[:, dt, :], in_=kv_f32)
                nc.sync.dma_start_transpose(khT[:, dt], kh_nat[:, dt, :])
                if g1_split:
                    kl_st = stage16.tile([128, N], f16, tag="stage16")
                    nc.vector.tensor_tensor(
                        out=kl_st, in0=kv_f32, in1=kh_nat[:, dt, :], op=Alu.subtract
                    )
                    nc.sync.dma_start_transpose(klT[:, dt], kl_st)

            for ct in range(CT):
                # ---- q prep for this c-tile ----
                q_f32 = raw.tile([128, N], f32, tag="raw")
                nc.sync.dma_start(out=q_f32, in_=xp[b, ct * 128 : (ct + 1) * 128, :])
                qh_st = stage16.tile([128, N], f16, tag="stage16")
                nc.gpsimd.tensor_copy(out=qh_st, in_=q_f32)
                qhT = qtpool.tile([128, NJ, 128], f16, tag="qhT")
                nc.sync.dma_start_transpose(qhT, qh_st)
                if g1_split:
                    ql_st = stage16.tile([128, N], f16, tag="stage16")
                    nc.vector.tensor_tensor(
                        out=ql_st, in0=q_f32, in1=qh_st, op=Alu.subtract
                    )
                    qlT = qtpool.tile([128, NJ, 128], f16, tag="qlT")
                    nc.sync.dma_start_transpose(qlT, ql_st)

                # ---- gemm1: energy[c_tile, :] accumulated over n-chunks ----
                e_ps = ps_e.tile([128, C], f32, tag="ps_e")
                for j in range(NJ):
                    last = j == NJ - 1
                    nc.tensor.matmul(
                        e_ps,
                        qhT[:, j, :],
                        khT[:, :, j, :],
                        start=(j == 0),
                        stop=(last and not g1_split),
                    )
                    if g1_split:
                        nc.tensor.matmul(
                            e_ps, qlT[:, j, :], khT[:, :, j, :], start=False, stop=False
                        )
                        nc.tensor.matmul(
                            e_ps, qhT[:, j, :], klT[:, :, j, :], start=False, stop=last
                        )

                # ---- softmax over d (free axis): att = exp(min - E)/sum ----
                min_t = small.tile([128, 1], f32, tag="min")
                nc.vector.tensor_reduce(min_t, e_ps, axis=Axis.X, op=Alu.min)
                att16 = att_pool.tile([128, C], f16, tag="att16")
                den = small.tile([128, 1], f32, tag="den")
                nc.scalar.activation(
                    out=att16,
                    in_=e_ps,
                    func=Act.Exp,
                    bias=min_t,
                    scale=-1.0,
                    accum_out=den,
                )
                rden = small.tile([128, 1], f32, tag="rden")
                nc.vector.reciprocal(rden, den)
                nc.vector.tensor_scalar_mul(att16, att16, rden)
                attT = att_pool.tile([128, CT, 128], f16, tag="attT")
                nc.sync.dma_start_transpose(attT, att16)

                # ---- gemm2 + final softmax over W segments ----
                for nj in range(NO):
                    o_ps = ps_o.tile([128, 512], f32, tag="ps_o")
                    for dt in range(CT):
                        nc.tensor.matmul(
                            o_ps,
                            attT[:, dt, :],
                            kh_nat[:, dt, nj * 512 : (nj + 1) * 512],
                            start=(dt == 0),
                            stop=(dt == CT - 1),
                        )
                    seg = o_ps.rearrange("p (s w) -> p s w", w=W)
                    nc.scalar.activation(out=seg, in_=seg, func=Act.Exp)
                    ssum = small.tile([128, 512 // W], f32, tag="ssum")
                    nc.vector.tensor_reduce(ssum, seg, axis=Axis.X, op=Alu.add)
                    rsum = small.tile([128, 512 // W], f32, tag="rsum")
                    nc.vector.reciprocal(rsum, ssum)
                    o_sb = opool.tile([128, 512 // W, W], f32, tag="osb")
                    nc.vector.tensor_tensor(
                        out=o_sb,
                        in0=seg,
                        in1=rsum[:, :, None].to_broadcast(seg.shape),
                        op=Alu.mult,
                    )
                    nc.sync.dma_start(
                        out=out[
                            b, ct * 128 : (ct + 1) * 128, nj * 512 : (nj + 1) * 512
                        ],
                        in_=o_sb,
                    )

    nc.finalize()
    return nc


def kernel(x_training: np.ndarray, x_pre: np.ndarray) -> np.ndarray:
    from concourse.bass_utils import run_bass_kernel_spmd

    nc = build_program()

    xt = np.ascontiguousarray(
        x_training.reshape(B, C, N).astype(np.float32, copy=False)
    )
    xp = np.ascontiguousarray(x_pre.reshape(B, C, N).astype(np.float32, copy=False))

    in_maps = []
    for i in range(N_CORES):
        sl = slice(i * B_PER_CORE, (i + 1) * B_PER_CORE)
        in_maps.append({"xt": xt[sl], "xp": xp[sl]})

    res = run_bass_kernel_spmd(nc, in_maps, list(range(N_CORES)))
    outs = [np.asarray(r["out"]) for r in res.results]
    return np.concatenate(outs, axis=0).reshape(B, C, H, W).astype(np.float32)
